# revision 10
# baseline (speedup 1.0000x reference)
"""Trainium2 Bass kernel for nn_CrossAttention (4-layer MLP -> cross-attention).

Sharding: data-parallel across batch B=8, one batch element per NeuronCore.

Layout strategy (per core):
  - activations flow feature-major (transposed): the MLP chain
    h_{l+1}^T = W_l^T @ h_l^T needs no transposes (W natural [K,M] = lhsT),
  - v is computed token-major (y^T tiles as stationary, Wv moving),
  - scores are computed transposed (scores^T = k @ q^T, kv on partitions) so
    E = exp(scores^T) feeds the attention output matmul directly as the
    stationary operand: out[q,D] = lhsT(E[kv,q]).T @ v[kv,D],
  - softmax skips max-subtraction (exact by shift invariance; scores ~ +-0.1),
    row sums come from an extra N=1 ones-column matmul accumulated alongside,
    normalization + bias bv applied on the fp32 output at the end.

Precision: the x-MLP, q, k and the scores matmul run in fp8(e4m3) DoubleRow
(2 contraction rows per PE pass -> 2x rate); their errors only perturb the
attention weights, which average over Skv=2048, so the output impact is
negligible (measured: 4.0e-3 rel vs 3.3e-3 for all-bf16). v and attn@v stay
bf16 (E ~ 1 +- 0.08 and v feed the output directly; fp8 would destroy the
signal). All accumulation fp32 in PSUM.

fp8 operands are pair-packed for DoubleRow: logical feature k = (2t+r)*128+p
lives in tile t, partition p, middle index r, i.e. SBUF tiles [128, 2, N]
(weights pre-packed on host to [K/2, 2*N] so each tile is one contiguous DMA).

Everything is SBUF-resident (no DRAM round-trips); y/Wk/Wv prefetch from t=0.
"""

import sys

if "/opt/trn_rl_repo" not in sys.path:
    sys.path.insert(0, "/opt/trn_rl_repo")

import numpy as np
import ml_dtypes

P = 128
D = 1024
DB = 512
S = 2048
KD = D // P       # 8 feature tiles of 128
KB = DB // P      # 4
PD = KD // 2      # 4 fp8 pair-tiles for a 1024 contraction
PB = KB // 2      # 2 for 512
NT = S // P       # 16 token tiles
NB = 512          # moving-operand free-dim block
NBLK = S // NB    # 4 token blocks
HALF = S // 2     # q processed in 2 halves during attention
NCORES = 8
SCALE = float(1.0 / np.sqrt(D))

BF16 = ml_dtypes.bfloat16
FP8 = ml_dtypes.float8_e4m3

_NC = None


def build_nc():
    """Build + compile the per-core Bass program (cached)."""
    global _NC
    if _NC is not None:
        return _NC

    from contextlib import ExitStack
    import concourse.bass as bass
    import concourse.tile as tile
    from concourse import bacc, mybir

    BF = mybir.dt.bfloat16
    F8 = mybir.dt.float8e4
    F32 = mybir.dt.float32
    AF = mybir.ActivationFunctionType
    DR = mybir.MatmulPerfMode.DoubleRow

    nc = bacc.Bacc("TRN2", target_bir_lowering=False, debug=False,
                   num_devices=NCORES)

    def din(name, shape, dt):
        return nc.dram_tensor(name, shape, dt, kind="ExternalInput").ap()

    # fp8 operands arrive pair-packed: [K/2, 2*N]
    x8d = din("x8", [D // 2, 2 * S], F8)
    y8d = din("y8", [D // 2, 2 * S], F8)
    yTd = din("yT", [D, S], BF)
    W1d = din("W1", [D // 2, 2 * D], F8)
    W2d = din("W2", [D // 2, 2 * DB], F8)
    W3d = din("W3", [DB // 2, 2 * D], F8)
    W4d = din("W4", [D // 2, 2 * D], F8)
    Wqd = din("Wq", [D // 2, 2 * D], F8)
    Wkd = din("Wk", [D // 2, 2 * D], F8)
    Wvd = din("Wv", [D, D], BF)
    b1 = din("b1", [P, KD], F32)
    b2 = din("b2", [P, KB], F32)
    b3 = din("b3", [P, KD], F32)
    b4 = din("b4", [P, KD], F32)
    bq = din("bq", [P, KD], F32)
    bk = din("bk", [P, KD], F32)
    bv = din("bv", [D], F32)
    out = nc.dram_tensor("out", [S, D], F32, kind="ExternalOutput").ap()

    with tile.TileContext(nc) as tc, ExitStack() as ctx:
        small = ctx.enter_context(tc.tile_pool(name="small", bufs=1))
        rpool = ctx.enter_context(tc.tile_pool(name="rpool", bufs=4))
        outp = ctx.enter_context(tc.tile_pool(name="outp", bufs=2))

        def load_bias(src, cols, tag):
            t = small.tile([P, cols], F32, tag=tag, name=tag)
            nc.gpsimd.dma_start(out=t, in_=src)
            return t

        b1_sb = load_bias(b1, KD, "b1")
        b2_sb = load_bias(b2, KB, "b2")
        b3_sb = load_bias(b3, KD, "b3")
        b4_sb = load_bias(b4, KD, "b4")
        bq_sb = load_bias(bq, KD, "bq")
        bk_sb = load_bias(bk, KD, "bk")

        # bv replicated across partitions for the final (exact, fp32) bias add
        bv_rep = small.tile([P, D], F32, tag="bvrep", name="bvrep")
        bv_bcast = bass.AP(tensor=bv.tensor, offset=bv.offset,
                           ap=[[0, P]] + list(bv.ap))
        nc.gpsimd.dma_start(out=bv_rep, in_=bv_bcast)

        ones_col = small.tile([P, 1], BF, tag="ones", name="ones")
        nc.vector.memset(ones_col, 1.0)

        def alloc_pairs(pool, pairs, n, tag, dt=F8):
            """fp8 pair-packed tiles [P, 2, n]."""
            return [pool.tile([P, 2, n], dt, tag=f"{tag}{t}", name=f"{tag}{t}")
                    for t in range(pairs)]

        def load_pairs(tiles, src, n):
            for t, tl in enumerate(tiles):
                nc.sync.dma_start(
                    out=tl,
                    in_=src[t * P:(t + 1) * P, :].rearrange(
                        "p (r s) -> p r s", r=2))

        def alloc_rows(pool, ktiles, n, tag, dt=BF):
            return [pool.tile([P, n], dt, tag=f"{tag}{k}", name=f"{tag}{k}")
                    for k in range(ktiles)]

        def fm_layer8(psum, src8, w8, pairs, mtiles, bias_sb, func, dst8):
            """fp8 DoubleRow feature-major layer into pair-packed fp8 dst.

            dst8 may also be a list of plain [P, S] tiles (dst_packed=False
            semantics chosen by tile rank)."""
            for m in range(mtiles):
                psg = psum.tile([P, S], F32, tag="mm", name="mm")
                for t in range(pairs):
                    lhs = w8[t][:, :, m * P:(m + 1) * P]
                    for tb in range(NBLK):
                        nc.tensor.matmul(psg[:, tb * NB:(tb + 1) * NB], lhs,
                                         src8[t][:, :, tb * NB:(tb + 1) * NB],
                                         start=(t == 0), stop=(t == pairs - 1),
                                         perf_mode=DR)
                nc.scalar.activation(dst8[m // 2][:, m % 2, :], psg, func,
                                     bias=bias_sb[:, m:m + 1], scale=1.0)

        # ------ persistent attention operands (q8, k8, v) + y prefetch ------
        with tc.tile_pool(name="pq", bufs=1) as pq, \
             tc.tile_pool(name="pk", bufs=1) as pk, \
             tc.tile_pool(name="pvp", bufs=1) as pvp, \
             tc.tile_pool(name="py", bufs=1) as py, \
             tc.tile_pool(name="pwk", bufs=1) as pwk:
            q8 = alloc_pairs(pq, PD, S, "q8")
            k8 = alloc_pairs(pk, PD, S, "k8")
            vs = [pvp.tile([P, D], BF, tag=f"v{t}", name=f"v{t}")
                  for t in range(NT)]
            y8 = alloc_pairs(py, PD, S, "y8")
            wk8 = alloc_pairs(pwk, PD, D, "wk8")

            # ---------------- Stage A: x-MLP -> q8 (in SBUF) ----------------
            with tc.tile_pool(name="wx", bufs=1) as wx, \
                 tc.tile_pool(name="px", bufs=1) as px, \
                 tc.tile_pool(name="phA", bufs=1) as phA, \
                 tc.tile_pool(name="phB", bufs=1) as phB, \
                 tc.tile_pool(name="psA", bufs=2, space="PSUM") as psA:
                x8 = alloc_pairs(px, PD, S, "x8")
                w18 = alloc_pairs(wx, PD, D, "w18")
                # first-needed tiles first: interleave x8 / W1 pair loads
                for t in range(PD):
                    nc.sync.dma_start(
                        out=x8[t], in_=x8d[t * P:(t + 1) * P, :].rearrange(
                            "p (r s) -> p r s", r=2))
                    nc.sync.dma_start(
                        out=w18[t], in_=W1d[t * P:(t + 1) * P, :].rearrange(
                            "p (r s) -> p r s", r=2))
                w28 = alloc_pairs(wx, PD, DB, "w28")
                load_pairs(w28, W2d, DB)
                w38 = alloc_pairs(wx, PB, D, "w38")
                load_pairs(w38, W3d, D)
                w48 = alloc_pairs(wx, PD, D, "w48")
                load_pairs(w48, W4d, D)
                wq8 = alloc_pairs(wx, PD, D, "wq8")
                load_pairs(wq8, Wqd, D)
                # y-side prefetch (queued behind stage A's needs)
                load_pairs(y8, y8d, S)
                load_pairs(wk8, Wkd, D)

                h18 = alloc_pairs(phA, PD, S, "ha")
                h28 = alloc_pairs(phB, PB, S, "hb")
                h38 = alloc_pairs(phA, PD, S, "ha")   # reuse phA slots
                h48 = alloc_pairs(phB, PD, S, "hb")   # grow phB to 4 pair slots
                fm_layer8(psA, x8, w18, PD, KD, b1_sb, AF.Relu, h18)
                fm_layer8(psA, h18, w28, PD, KB, b2_sb, AF.Relu, h28)
                fm_layer8(psA, h28, w38, PB, KD, b3_sb, AF.Relu, h38)
                fm_layer8(psA, h38, w48, PD, KD, b4_sb, AF.Relu, h48)
                fm_layer8(psA, h48, wq8, PD, KD, bq_sb, AF.Identity, q8)

            # ------------ Stage B: y -> k8 (fp8), v (bf16, SBUF) ------------
            with tc.tile_pool(name="pwv", bufs=1) as pwv, \
                 tc.tile_pool(name="psBk", bufs=1, space="PSUM") as psBk, \
                 tc.tile_pool(name="psBv", bufs=2, space="PSUM") as psBv:
                # yT (bf16, for v) + Wv loads hide under the k-phase compute
                ys = alloc_rows(pwv, KD, S, "y")
                for k in range(KD):
                    nc.sync.dma_start(out=ys[k], in_=yTd[k * P:(k + 1) * P, :])
                wvs = alloc_rows(pwv, KD, D, "wv")
                for k in range(KD):
                    nc.sync.dma_start(out=wvs[k], in_=Wvd[k * P:(k + 1) * P, :])
                # k^T in fp8 pairs (feature-major, bias per-partition)
                for m in range(KD):
                    psg = psBk.tile([P, S], F32, tag="kk", name="kk")
                    for t in range(PD):
                        lhs = wk8[t][:, :, m * P:(m + 1) * P]
                        for tb in range(NBLK):
                            nc.tensor.matmul(psg[:, tb * NB:(tb + 1) * NB],
                                             lhs,
                                             y8[t][:, :, tb * NB:(tb + 1) * NB],
                                             start=(t == 0), stop=(t == PD - 1),
                                             perf_mode=DR)
                    nc.scalar.activation(k8[m // 2][:, m % 2, :], psg,
                                         AF.Identity,
                                         bias=bk_sb[:, m:m + 1], scale=1.0)
                # v (token-major bf16; bias-free — bv folded into the final add)
                for tq in range(NT):
                    pv_ = psBv.tile([P, D], F32, tag="vv", name="vv")
                    for k in range(KD):
                        lhs = ys[k][:, tq * P:(tq + 1) * P]
                        for nb2 in range(2):
                            nc.tensor.matmul(pv_[:, nb2 * NB:(nb2 + 1) * NB],
                                             lhs,
                                             wvs[k][:, nb2 * NB:(nb2 + 1) * NB],
                                             start=(k == 0), stop=(k == KD - 1))
                    nc.vector.tensor_add(vs[tq], pv_, bv_rep)

            # ---------------- Stage C: attention ----------------
            with tc.tile_pool(name="pE", bufs=2) as pE, \
                 tc.tile_pool(name="psCs", bufs=3, space="PSUM") as psCs, \
                 tc.tile_pool(name="psCo", bufs=2, space="PSUM") as psCo, \
                 tc.tile_pool(name="psCS", bufs=1, space="PSUM") as psCS:
                for half in range(2):
                    qoff = half * HALF
                    # E^T = exp(scale * k @ q^T) for this half of q columns
                    ets = []
                    for tk in range(NT):
                        et = pE.tile([P, HALF], BF, tag=f"e{tk}", name=f"e{tk}")
                        for qb in range(HALF // NB):
                            ps = psCs.tile([P, NB], F32, tag="sc", name="sc")
                            for t in range(PD):
                                nc.tensor.matmul(
                                    ps, k8[t][:, :, tk * P:(tk + 1) * P],
                                    q8[t][:, :,
                                          qoff + qb * NB:qoff + (qb + 1) * NB],
                                    start=(t == 0), stop=(t == PD - 1),
                                    perf_mode=DR)
                            nc.scalar.activation(et[:, qb * NB:(qb + 1) * NB],
                                                 ps, AF.Exp, bias=0.0,
                                                 scale=SCALE)
                        ets.append(et)
                    # out rows for this half: unnormalized E @ v plus row sums
                    for tq8 in range(HALF // P):
                        tq = half * (HALF // P) + tq8
                        po = psCo.tile([P, D], F32, tag="oo", name="oo")
                        pS = psCS.tile([P, 1], F32, tag="ss", name="ss")
                        for tk in range(NT):
                            lhs = ets[tk][:, tq8 * P:(tq8 + 1) * P]
                            nc.tensor.matmul(po[:, 0:NB], lhs, vs[tk][:, 0:NB],
                                             start=(tk == 0),
                                             stop=(tk == NT - 1))
                            nc.tensor.matmul(po[:, NB:D], lhs, vs[tk][:, NB:D],
                                             start=(tk == 0),
                                             stop=(tk == NT - 1))
                            nc.tensor.matmul(pS, lhs, ones_col,
                                             start=(tk == 0),
                                             stop=(tk == NT - 1))
                        rinv = rpool.tile([P, 1], F32, tag="ri", name="ri")
                        nc.vector.reciprocal(rinv, pS)
                        ot = outp.tile([P, D], F32, tag="ot", name="ot")
                        nc.vector.tensor_scalar_mul(ot, po, rinv)
                        nc.sync.dma_start(out=out[tq * P:(tq + 1) * P, :],
                                          in_=ot)

    nc.compile()
    _NC = nc
    return nc


def _pack8(w):
    """[K, N] -> DoubleRow pair-packed fp8 [K/2, 2N]:
    out[t*128+p, r*N+m] = w[(2t+r)*128+p, m]."""
    K, N = w.shape
    return np.ascontiguousarray(
        w.astype(FP8).reshape(K // 256, 2, 128, N)
        .transpose(0, 2, 1, 3).reshape(K // 2, 2 * N))


def make_in_maps(inputs):
    """Host-side prep: per-core batch shard, fp8/bf16 casts + pair packing,
    feature-major transposes of x/y, bias relayout."""
    x = np.asarray(inputs["x"])
    y = np.asarray(inputs["y"])
    shared = {}
    for k in ("W1", "W2", "W3", "W4", "Wq", "Wk"):
        shared[k] = _pack8(np.asarray(inputs[k]).astype(np.float32))
    shared["Wv"] = np.ascontiguousarray(np.asarray(inputs["Wv"]).astype(BF16))
    for k, nt in (("b1", KD), ("b2", KB), ("b3", KD), ("b4", KD),
                  ("bq", KD), ("bk", KD)):
        shared[k] = np.ascontiguousarray(
            np.asarray(inputs[k]).astype(np.float32).reshape(nt, P).T)
    shared["bv"] = np.ascontiguousarray(
        np.asarray(inputs["bv"]).astype(np.float32).reshape(D))
    in_maps = []
    for b in range(x.shape[0]):
        m = dict(shared)
        xT = np.ascontiguousarray(x[b].T)
        yT = np.ascontiguousarray(y[b].T)
        m["x8"] = _pack8(xT)
        m["y8"] = _pack8(yT)
        m["yT"] = yT.astype(BF16)
        in_maps.append(m)
    return in_maps


def kernel(**inputs):
    from concourse.bass_utils import run_bass_kernel_spmd

    nc = build_nc()
    in_maps = make_in_maps(inputs)
    res = run_bass_kernel_spmd(nc, in_maps, list(range(len(in_maps))))
    return np.stack([np.asarray(r["out"], dtype=np.float32)
                     for r in res.results])


# revision 11
# speedup vs baseline: 1.0322x; 1.0322x over previous
"""Trainium2 Bass kernel for nn_CrossAttention (4-layer MLP -> cross-attention).

Sharding: data-parallel across batch B=8, one batch element per NeuronCore.

Layout strategy (per core):
  - activations flow feature-major (transposed): the MLP chain
    h_{l+1}^T = W_l^T @ h_l^T needs no transposes (W natural [K,M] = lhsT),
  - v is computed token-major (y^T tiles as stationary, Wv moving),
  - scores are computed transposed (scores^T = k @ q^T, kv on partitions) so
    E = exp(scores^T) feeds the attention output matmul directly as the
    stationary operand: out[q,D] = lhsT(E[kv,q]).T @ v[kv,D],
  - softmax skips max-subtraction (exact by shift invariance; scores ~ +-0.1),
    row sums come from an extra N=1 ones-column matmul accumulated alongside,
    normalization + bias bv applied on the fp32 output at the end.

Precision: the x-MLP, q, k and the scores matmul run in fp8(e4m3) DoubleRow
(2 contraction rows per PE pass -> 2x rate); their errors only perturb the
attention weights, which average over Skv=2048, so the output impact is
negligible (measured: 4.0e-3 rel vs 3.3e-3 for all-bf16). v and attn@v stay
bf16 (E ~ 1 +- 0.08 and v feed the output directly; fp8 would destroy the
signal). All accumulation fp32 in PSUM.

fp8 operands are pair-packed for DoubleRow: logical feature k = (2t+r)*128+p
lives in tile t, partition p, middle index r, i.e. SBUF tiles [128, 2, N]
(weights pre-packed on host to [K/2, 2*N] so each tile is one contiguous DMA).

Everything is SBUF-resident (no DRAM round-trips); y/Wk/Wv prefetch from t=0.
"""

import sys

if "/opt/trn_rl_repo" not in sys.path:
    sys.path.insert(0, "/opt/trn_rl_repo")

import numpy as np
import ml_dtypes

P = 128
D = 1024
DB = 512
S = 2048
KD = D // P       # 8 feature tiles of 128
KB = DB // P      # 4
PD = KD // 2      # 4 fp8 pair-tiles for a 1024 contraction
PB = KB // 2      # 2 for 512
NT = S // P       # 16 token tiles
NB = 512          # moving-operand free-dim block
NBLK = S // NB    # 4 token blocks
HALF = S // 2     # q processed in 2 halves during attention
NCORES = 8
SCALE = float(1.0 / np.sqrt(D))

BF16 = ml_dtypes.bfloat16
FP8 = ml_dtypes.float8_e4m3

_NC = None


def build_nc():
    """Build + compile the per-core Bass program (cached)."""
    global _NC
    if _NC is not None:
        return _NC

    from contextlib import ExitStack
    import concourse.bass as bass
    import concourse.tile as tile
    from concourse import bacc, mybir

    BF = mybir.dt.bfloat16
    F8 = mybir.dt.float8e4
    F32 = mybir.dt.float32
    AF = mybir.ActivationFunctionType
    DR = mybir.MatmulPerfMode.DoubleRow

    nc = bacc.Bacc("TRN2", target_bir_lowering=False, debug=False,
                   num_devices=NCORES)

    def din(name, shape, dt):
        return nc.dram_tensor(name, shape, dt, kind="ExternalInput").ap()

    # fp8 operands arrive pair-packed: [K/2, 2*N]
    x8d = din("x8", [D // 2, 2 * S], F8)
    y8d = din("y8", [D // 2, 2 * S], F8)
    yTd = din("yT", [D, S], BF)
    W1d = din("W1", [D // 2, 2 * D], F8)
    W2d = din("W2", [D // 2, 2 * DB], F8)
    W3d = din("W3", [DB // 2, 2 * D], F8)
    W4d = din("W4", [D // 2, 2 * D], F8)
    Wqd = din("Wq", [D // 2, 2 * D], F8)
    Wkd = din("Wk", [D // 2, 2 * D], F8)
    Wvd = din("Wv", [D, D], BF)
    b1 = din("b1", [P, KD], F32)
    b2 = din("b2", [P, KB], F32)
    b3 = din("b3", [P, KD], F32)
    b4 = din("b4", [P, KD], F32)
    bq = din("bq", [P, KD], F32)
    bk = din("bk", [P, KD], F32)
    bv = din("bv", [D], F32)
    out = nc.dram_tensor("out", [S, D], F32, kind="ExternalOutput").ap()

    with tile.TileContext(nc) as tc, ExitStack() as ctx:
        small = ctx.enter_context(tc.tile_pool(name="small", bufs=1))
        rpool = ctx.enter_context(tc.tile_pool(name="rpool", bufs=4))
        outp = ctx.enter_context(tc.tile_pool(name="outp", bufs=2))

        def load_bias(src, cols, tag):
            t = small.tile([P, cols], F32, tag=tag, name=tag)
            nc.gpsimd.dma_start(out=t, in_=src)
            return t

        b1_sb = load_bias(b1, KD, "b1")
        b2_sb = load_bias(b2, KB, "b2")
        b3_sb = load_bias(b3, KD, "b3")
        b4_sb = load_bias(b4, KD, "b4")
        bq_sb = load_bias(bq, KD, "bq")
        bk_sb = load_bias(bk, KD, "bk")

        # bv replicated across partitions for the final (exact, fp32) bias add
        bv_rep = small.tile([P, D], F32, tag="bvrep", name="bvrep")
        bv_bcast = bass.AP(tensor=bv.tensor, offset=bv.offset,
                           ap=[[0, P]] + list(bv.ap))
        nc.gpsimd.dma_start(out=bv_rep, in_=bv_bcast)

        ones_col = small.tile([P, 1], BF, tag="ones", name="ones")
        nc.vector.memset(ones_col, 1.0)

        def alloc_pairs(pool, pairs, n, tag, dt=F8):
            """fp8 pair-packed tiles [P, 2, n]."""
            return [pool.tile([P, 2, n], dt, tag=f"{tag}{t}", name=f"{tag}{t}")
                    for t in range(pairs)]

        def load_pairs(tiles, src, n):
            for t, tl in enumerate(tiles):
                nc.sync.dma_start(
                    out=tl,
                    in_=src[t * P:(t + 1) * P, :].rearrange(
                        "p (r s) -> p r s", r=2))

        def alloc_rows(pool, ktiles, n, tag, dt=BF):
            return [pool.tile([P, n], dt, tag=f"{tag}{k}", name=f"{tag}{k}")
                    for k in range(ktiles)]

        def fm_layer8(psum, src8, w8, pairs, mtiles, bias_sb, func, dst8):
            """fp8 DoubleRow feature-major layer into pair-packed fp8 dst.

            dst8 may also be a list of plain [P, S] tiles (dst_packed=False
            semantics chosen by tile rank)."""
            for m in range(mtiles):
                pss = [psum.tile([P, NB], F32, tag="mm", name="mm")
                       for _ in range(NBLK)]
                for t in range(pairs):
                    lhs = w8[t][:, :, m * P:(m + 1) * P]
                    for tb in range(NBLK):
                        nc.tensor.matmul(pss[tb], lhs,
                                         src8[t][:, :, tb * NB:(tb + 1) * NB],
                                         start=(t == 0), stop=(t == pairs - 1),
                                         perf_mode=DR)
                for tb in range(NBLK):
                    dst = dst8[m // 2][:, m % 2, tb * NB:(tb + 1) * NB]
                    nc.scalar.activation(dst, pss[tb], func,
                                         bias=bias_sb[:, m:m + 1], scale=1.0)

        # ------ persistent attention operands (q8, k8, v) + y prefetch ------
        with tc.tile_pool(name="pq", bufs=1) as pq, \
             tc.tile_pool(name="pk", bufs=1) as pk, \
             tc.tile_pool(name="pvp", bufs=1) as pvp, \
             tc.tile_pool(name="py", bufs=1) as py, \
             tc.tile_pool(name="pwk", bufs=1) as pwk:
            q8 = alloc_pairs(pq, PD, S, "q8")
            k8 = alloc_pairs(pk, PD, S, "k8")
            vs = [pvp.tile([P, D], BF, tag=f"v{t}", name=f"v{t}")
                  for t in range(NT)]
            y8 = alloc_pairs(py, PD, S, "y8")
            wk8 = alloc_pairs(pwk, PD, D, "wk8")

            # ---------------- Stage A: x-MLP -> q8 (in SBUF) ----------------
            with tc.tile_pool(name="wx", bufs=1) as wx, \
                 tc.tile_pool(name="px", bufs=1) as px, \
                 tc.tile_pool(name="phA", bufs=1) as phA, \
                 tc.tile_pool(name="phB", bufs=1) as phB, \
                 tc.tile_pool(name="psA", bufs=8, space="PSUM") as psA:
                x8 = alloc_pairs(px, PD, S, "x8")
                w18 = alloc_pairs(wx, PD, D, "w18")
                # first-needed tiles first: interleave x8 / W1 pair loads
                for t in range(PD):
                    nc.sync.dma_start(
                        out=x8[t], in_=x8d[t * P:(t + 1) * P, :].rearrange(
                            "p (r s) -> p r s", r=2))
                    nc.sync.dma_start(
                        out=w18[t], in_=W1d[t * P:(t + 1) * P, :].rearrange(
                            "p (r s) -> p r s", r=2))
                w28 = alloc_pairs(wx, PD, DB, "w28")
                load_pairs(w28, W2d, DB)
                w38 = alloc_pairs(wx, PB, D, "w38")
                load_pairs(w38, W3d, D)
                w48 = alloc_pairs(wx, PD, D, "w48")
                load_pairs(w48, W4d, D)
                wq8 = alloc_pairs(wx, PD, D, "wq8")
                load_pairs(wq8, Wqd, D)
                # y-side prefetch (queued behind stage A's needs)
                load_pairs(y8, y8d, S)
                load_pairs(wk8, Wkd, D)

                h18 = alloc_pairs(phA, PD, S, "ha")
                h28 = alloc_pairs(phB, PB, S, "hb")
                h38 = alloc_pairs(phA, PD, S, "ha")   # reuse phA slots
                h48 = alloc_pairs(phB, PD, S, "hb")   # grow phB to 4 pair slots
                fm_layer8(psA, x8, w18, PD, KD, b1_sb, AF.Relu, h18)
                fm_layer8(psA, h18, w28, PD, KB, b2_sb, AF.Relu, h28)
                fm_layer8(psA, h28, w38, PB, KD, b3_sb, AF.Relu, h38)
                fm_layer8(psA, h38, w48, PD, KD, b4_sb, AF.Relu, h48)
                fm_layer8(psA, h48, wq8, PD, KD, bq_sb, AF.Identity, q8)

            # ------------ Stage B: y -> k8 (fp8), v (bf16, SBUF) ------------
            with tc.tile_pool(name="pwv", bufs=1) as pwv, \
                 tc.tile_pool(name="psBk", bufs=4, space="PSUM") as psBk, \
                 tc.tile_pool(name="psBv", bufs=2, space="PSUM") as psBv:
                # yT (bf16, for v) + Wv loads hide under the k-phase compute
                ys = alloc_rows(pwv, KD, S, "y")
                for k in range(KD):
                    nc.sync.dma_start(out=ys[k], in_=yTd[k * P:(k + 1) * P, :])
                wvs = alloc_rows(pwv, KD, D, "wv")
                for k in range(KD):
                    nc.sync.dma_start(out=wvs[k], in_=Wvd[k * P:(k + 1) * P, :])
                # k^T in fp8 pairs (feature-major, bias per-partition)
                for m in range(KD):
                    pss = [psBk.tile([P, NB], F32, tag="kk", name="kk")
                           for _ in range(NBLK)]
                    for t in range(PD):
                        lhs = wk8[t][:, :, m * P:(m + 1) * P]
                        for tb in range(NBLK):
                            nc.tensor.matmul(pss[tb], lhs,
                                             y8[t][:, :, tb * NB:(tb + 1) * NB],
                                             start=(t == 0), stop=(t == PD - 1),
                                             perf_mode=DR)
                    for tb in range(NBLK):
                        nc.scalar.activation(
                            k8[m // 2][:, m % 2, tb * NB:(tb + 1) * NB],
                            pss[tb], AF.Identity,
                            bias=bk_sb[:, m:m + 1], scale=1.0)
                # v (token-major bf16; bias-free — bv folded into the final add)
                for tq in range(NT):
                    pv_ = psBv.tile([P, D], F32, tag="vv", name="vv")
                    for k in range(KD):
                        lhs = ys[k][:, tq * P:(tq + 1) * P]
                        for nb2 in range(2):
                            nc.tensor.matmul(pv_[:, nb2 * NB:(nb2 + 1) * NB],
                                             lhs,
                                             wvs[k][:, nb2 * NB:(nb2 + 1) * NB],
                                             start=(k == 0), stop=(k == KD - 1))
                    nc.vector.tensor_add(vs[tq], pv_, bv_rep)

            # ---------------- Stage C: attention ----------------
            with tc.tile_pool(name="pE", bufs=2) as pE, \
                 tc.tile_pool(name="psCs", bufs=3, space="PSUM") as psCs, \
                 tc.tile_pool(name="psCo", bufs=2, space="PSUM") as psCo, \
                 tc.tile_pool(name="psCS", bufs=1, space="PSUM") as psCS:
                for half in range(2):
                    qoff = half * HALF
                    # E^T = exp(scale * k @ q^T) for this half of q columns
                    ets = []
                    for tk in range(NT):
                        et = pE.tile([P, HALF], BF, tag=f"e{tk}", name=f"e{tk}")
                        for qb in range(HALF // NB):
                            ps = psCs.tile([P, NB], F32, tag="sc", name="sc")
                            for t in range(PD):
                                nc.tensor.matmul(
                                    ps, k8[t][:, :, tk * P:(tk + 1) * P],
                                    q8[t][:, :,
                                          qoff + qb * NB:qoff + (qb + 1) * NB],
                                    start=(t == 0), stop=(t == PD - 1),
                                    perf_mode=DR)
                            nc.scalar.activation(et[:, qb * NB:(qb + 1) * NB],
                                                 ps, AF.Exp, bias=0.0,
                                                 scale=SCALE)
                        ets.append(et)
                    # out rows for this half: unnormalized E @ v plus row sums
                    for tq8 in range(HALF // P):
                        tq = half * (HALF // P) + tq8
                        po = psCo.tile([P, D], F32, tag="oo", name="oo")
                        pS = psCS.tile([P, 1], F32, tag="ss", name="ss")
                        for tk in range(NT):
                            lhs = ets[tk][:, tq8 * P:(tq8 + 1) * P]
                            nc.tensor.matmul(po[:, 0:NB], lhs, vs[tk][:, 0:NB],
                                             start=(tk == 0),
                                             stop=(tk == NT - 1))
                            nc.tensor.matmul(po[:, NB:D], lhs, vs[tk][:, NB:D],
                                             start=(tk == 0),
                                             stop=(tk == NT - 1))
                            nc.tensor.matmul(pS, lhs, ones_col,
                                             start=(tk == 0),
                                             stop=(tk == NT - 1))
                        rinv = rpool.tile([P, 1], F32, tag="ri", name="ri")
                        nc.vector.reciprocal(rinv, pS)
                        ot = outp.tile([P, D], F32, tag="ot", name="ot")
                        nc.vector.tensor_scalar_mul(ot, po, rinv)
                        nc.sync.dma_start(out=out[tq * P:(tq + 1) * P, :],
                                          in_=ot)

    nc.compile()
    _NC = nc
    return nc


def _pack8(w):
    """[K, N] -> DoubleRow pair-packed fp8 [K/2, 2N]:
    out[t*128+p, r*N+m] = w[(2t+r)*128+p, m]."""
    K, N = w.shape
    return np.ascontiguousarray(
        w.astype(FP8).reshape(K // 256, 2, 128, N)
        .transpose(0, 2, 1, 3).reshape(K // 2, 2 * N))


def make_in_maps(inputs):
    """Host-side prep: per-core batch shard, fp8/bf16 casts + pair packing,
    feature-major transposes of x/y, bias relayout."""
    x = np.asarray(inputs["x"])
    y = np.asarray(inputs["y"])
    shared = {}
    for k in ("W1", "W2", "W3", "W4", "Wq", "Wk"):
        shared[k] = _pack8(np.asarray(inputs[k]).astype(np.float32))
    shared["Wv"] = np.ascontiguousarray(np.asarray(inputs["Wv"]).astype(BF16))
    for k, nt in (("b1", KD), ("b2", KB), ("b3", KD), ("b4", KD),
                  ("bq", KD), ("bk", KD)):
        shared[k] = np.ascontiguousarray(
            np.asarray(inputs[k]).astype(np.float32).reshape(nt, P).T)
    shared["bv"] = np.ascontiguousarray(
        np.asarray(inputs["bv"]).astype(np.float32).reshape(D))
    in_maps = []
    for b in range(x.shape[0]):
        m = dict(shared)
        xT = np.ascontiguousarray(x[b].T)
        yT = np.ascontiguousarray(y[b].T)
        m["x8"] = _pack8(xT)
        m["y8"] = _pack8(yT)
        m["yT"] = yT.astype(BF16)
        in_maps.append(m)
    return in_maps


def kernel(**inputs):
    from concourse.bass_utils import run_bass_kernel_spmd

    nc = build_nc()
    in_maps = make_in_maps(inputs)
    res = run_bass_kernel_spmd(nc, in_maps, list(range(len(in_maps))))
    return np.stack([np.asarray(r["out"], dtype=np.float32)
                     for r in res.results])
